# revision 2
# baseline (speedup 1.0000x reference)
"""HMM log-likelihood (log-domain forward algorithm) on 8 Trainium2 cores.

Strategy: scaled linear-domain forward algorithm with warmup-halo sequence
parallelism.  The filtering distribution of an HMM forgets its initial
condition geometrically fast, so N=1e6 timesteps are split into 3840
independent chains (480/core); each chain starts from a uniform state W=20
steps before its owned region, then accumulates log-normalizers over its
owned L=260 steps.  Per core, chains are batched 4-wide across the 128 SBUF
partitions (block-diagonal T^T weights on the PE) with the chain-block index
in the matmul free dimension.  Host combines: exact prefix scan [0, W),
per-chain accumulated log-normalizers + residual log-sum of final states,
and an exact tail scan for the last ~1.6k steps.
"""

import sys

for p in ("/opt/trn_rl_repo", "/root/.axon_site", "/root/.axon_site/_ro/trn_rl_repo",
          "/root/.axon_site/_ro/pypackages"):
    if p not in sys.path:
        sys.path.insert(0, p)

import numpy as np

K = 32
N = 1_000_000
NCORES = 8
W = 20            # warmup (halo) steps per chain
L = 260           # owned steps per chain
CC = 480          # chains per core
SPAN = W + L      # 280 sequential steps
SBLK = 140        # timesteps per load window
NWIN = SPAN // SBLK
NB = CC // 4      # 120 four-chain blocks
G = 2             # interleaved compute groups
F = NB // G       # 60 blocks (matmul free dim) per group
RESC = 48         # rescale period after warmup
NSL = CC * L + W  # per-core input slice columns
COVERED = W + NCORES * CC * L

_cache = {}


def _build():
    import concourse.bass as bass
    import concourse.bacc as bacc
    import concourse.mybir as mybir
    import concourse.tile as tile
    from contextlib import ExitStack

    f32 = mybir.dt.float32
    AF = mybir.ActivationFunctionType

    nc = bacc.Bacc("TRN2", target_bir_lowering=False, debug=False,
                   num_devices=NCORES)
    x = nc.dram_tensor("x", [K, NSL], f32, kind="ExternalInput")
    wmat = nc.dram_tensor("wmat", [128, 128], f32, kind="ExternalInput")
    omat = nc.dram_tensor("omat", [128, 128], f32, kind="ExternalInput")
    acc_out = nc.dram_tensor("acc_out", [128, NB], f32, kind="ExternalOutput")
    fin_out = nc.dram_tensor("fin_out", [128, NB], f32, kind="ExternalOutput")

    resc_steps = {W - 1} | {W - 1 + RESC * j for j in range(1, SPAN // RESC + 1)
                            if W - 1 + RESC * j < SPAN}

    with tile.TileContext(nc) as tc:
        with ExitStack() as ctx:
            cpool = ctx.enter_context(tc.tile_pool(name="const", bufs=1))
            rpool = ctx.enter_context(tc.tile_pool(name="rp", bufs=NWIN))
            mpool = ctx.enter_context(tc.tile_pool(name="mp", bufs=2))
            pspool = ctx.enter_context(
                tc.tile_pool(name="ps", bufs=2, space=bass.MemorySpace.PSUM))

            w_t = cpool.tile([128, 128], f32, tag="w")
            nc.sync.dma_start(w_t[:], wmat[:])
            o_t = cpool.tile([128, 128], f32, tag="o")
            nc.sync.dma_start(o_t[:], omat[:])

            S, A = [], []
            for g in range(G):
                st = cpool.tile([128, F], f32, tag=f"S{g}")
                nc.vector.memset(st[:], 1.0)
                ac = cpool.tile([128, F], f32, tag=f"A{g}")
                nc.vector.memset(ac[:], 0.0)
                S.append(st)
                A.append(ac)

            # Load + exp windows.  R[g][w] layout: [128, F, SBLK], partition
            # p = 32*q + k holds chain (g*F + cb)*4 + q, state k.
            R = [[None] * NWIN for _ in range(G)]
            NCHUNK = 4
            CH = F // NCHUNK
            for w in range(NWIN):
                for g in range(G):
                    rt = rpool.tile([128, F, SBLK], f32, tag=f"R{g}")
                    for ch in range(NCHUNK):
                        cb0 = ch * CH
                        for q in range(4):
                            off = ((g * F + cb0) * 4 + q) * L + w * SBLK
                            src = bass.AP(x, off,
                                          [[NSL, 32], [4 * L, CH], [1, SBLK]])
                            nc.sync.dma_start(
                                rt[32 * q:32 * q + 32, cb0:cb0 + CH, :], src)
                    # exp in place, chunked along s so compute starts early
                    EC = 4
                    for ec in range(EC):
                        s0 = ec * (SBLK // EC)
                        nc.scalar.activation(
                            rt[:, :, s0:s0 + SBLK // EC],
                            rt[:, :, s0:s0 + SBLK // EC], AF.Exp)
                    R[g][w] = rt

            for s in range(SPAN):
                w, si = divmod(s, SBLK)
                for g in range(G):
                    ps = pspool.tile([128, F], f32, tag=f"mm{g}")
                    nc.tensor.matmul(ps[:], w_t[:], S[g][:], start=True, stop=True)
                    nc.vector.tensor_mul(S[g][:], ps[:], R[g][w][:, :, si])
                if s in resc_steps:
                    for g in range(G):
                        sm = pspool.tile([128, F], f32, tag=f"sm{g}")
                        nc.tensor.matmul(sm[:], o_t[:], S[g][:], start=True, stop=True)
                        rc = mpool.tile([128, F], f32, tag=f"rc{g}")
                        nc.vector.reciprocal(rc[:], sm[:])
                        nc.vector.tensor_mul(S[g][:], S[g][:], rc[:])
                        if s >= W:
                            lg = mpool.tile([128, F], f32, tag=f"lg{g}")
                            nc.scalar.activation(lg[:], sm[:], AF.Ln)
                            nc.vector.tensor_add(A[g][:], A[g][:], lg[:])

            for g in range(G):
                nc.sync.dma_start(acc_out[:, g * F:(g + 1) * F], A[g][:])
                nc.sync.dma_start(fin_out[:, g * F:(g + 1) * F], S[g][:])

    nc.compile()
    return nc


def _get_nc():
    if "nc" not in _cache:
        _cache["nc"] = _build()
    return _cache["nc"]


def _log_softmax64(v, axis):
    v = v.astype(np.float64)
    m = v.max(axis=axis, keepdims=True)
    e = np.exp(v - m)
    return v - m - np.log(e.sum(axis=axis, keepdims=True))


def kernel(log_pdf: np.ndarray, pi: np.ndarray, T: np.ndarray) -> np.ndarray:
    from concourse.bass_utils import run_bass_kernel_spmd

    log_pdf = np.ascontiguousarray(log_pdf, dtype=np.float32)
    log_pi64 = _log_softmax64(pi, 0)
    log_T64 = _log_softmax64(T, 1)
    T64 = np.exp(log_T64)                     # row-stochastic [K, K] f64
    T32 = T64.astype(np.float32)

    wm = np.zeros((128, 128), dtype=np.float32)
    om = np.zeros((128, 128), dtype=np.float32)
    for q in range(4):
        wm[32 * q:32 * q + 32, 32 * q:32 * q + 32] = T32.T
        om[32 * q:32 * q + 32, 32 * q:32 * q + 32] = 1.0
    in_maps = []
    for k in range(NCORES):
        c0 = k * CC * L
        in_maps.append({
            "x": np.ascontiguousarray(log_pdf[:, c0:c0 + NSL]),
            "wmat": wm,
            "omat": om,
        })

    nc = _get_nc()
    res = run_bass_kernel_spmd(nc, in_maps, list(range(NCORES))).results

    # ---- host combine (f64) ----
    LP = log_pdf  # f32 view; cast per-column below
    # exact prefix [0, W)
    a = np.exp(log_pi64 + LP[:, 0].astype(np.float64))
    c = a.sum()
    total = np.log(c)
    a /= c
    for t in range(1, W):
        a = np.exp(LP[:, t].astype(np.float64)) * (T64 @ a)
        c = a.sum()
        total += np.log(c)
        a /= c

    # per-chain contributions
    last_vec = None
    for k in range(NCORES):
        acc = res[k]["acc_out"]   # [128, NB]
        fin = res[k]["fin_out"]   # [128, NB]
        for g in range(G):
            for cb in range(F):
                for q in range(4):
                    col = g * F + cb
                    vec = fin[32 * q:32 * q + 32, col].astype(np.float64)
                    sv = vec.sum()
                    total += acc[32 * q, col].astype(np.float64) + np.log(sv)
                    if (k == NCORES - 1 and g == G - 1 and cb == F - 1
                            and q == 3):
                        last_vec = vec / sv

    # exact tail [COVERED, N)
    a = last_vec
    for t in range(COVERED, N):
        a = np.exp(LP[:, t].astype(np.float64)) * (T64 @ a)
        c = a.sum()
        total += np.log(c)
        a /= c

    return np.float32(total)


# revision 7
# speedup vs baseline: 1.5525x; 1.5525x over previous
"""HMM log-likelihood (log-domain forward algorithm) on 8 Trainium2 cores.

Strategy: scaled linear-domain forward algorithm with warmup-halo sequence
parallelism.  The filtering distribution of an HMM forgets its initial
condition geometrically fast, so N=1e6 timesteps are split into 3840
independent chains (480/core); each chain starts from a uniform state W=20
steps before its owned region, then accumulates log-normalizers over its
owned L=260 steps.  Per core, chains are batched 4-wide across the 128 SBUF
partitions (block-diagonal T^T weights on the PE) with the chain-block index
in the matmul free dimension.  Host combines: exact prefix scan [0, W),
per-chain accumulated log-normalizers + residual log-sum of final states,
and an exact tail scan for the last ~1.6k steps.
"""

import sys

for p in ("/opt/trn_rl_repo", "/root/.axon_site", "/root/.axon_site/_ro/trn_rl_repo",
          "/root/.axon_site/_ro/pypackages"):
    if p not in sys.path:
        sys.path.insert(0, p)

import numpy as np

K = 32
N = 1_000_000
NCORES = 8
W = 20            # warmup (halo) steps per chain
L = 260           # owned steps per chain
CC = 480          # chains per core
SPAN = W + L      # 280 sequential steps
SBLK = 140        # timesteps per load window
NWIN = SPAN // SBLK
NB = CC // 4      # 120 four-chain blocks
G = 2             # interleaved compute groups
F = NB // G       # 60 blocks (matmul free dim) per group
RESC = 48         # rescale period after warmup
NSL = CC * L + W  # per-core input slice columns
COVERED = W + NCORES * CC * L

_cache = {}


def _build():
    import concourse.bass as bass
    import concourse.bacc as bacc
    import concourse.mybir as mybir
    import concourse.tile as tile
    from contextlib import ExitStack

    f32 = mybir.dt.float32
    AF = mybir.ActivationFunctionType

    bf16 = mybir.dt.bfloat16

    nc = bacc.Bacc("TRN2", target_bir_lowering=False, debug=False,
                   num_devices=NCORES)
    x = nc.dram_tensor("x", [K, NSL], f32, kind="ExternalInput")
    wmat = nc.dram_tensor("wmat", [128, 128], bf16, kind="ExternalInput")
    omat = nc.dram_tensor("omat", [128, 128], bf16, kind="ExternalInput")
    ebias = nc.dram_tensor("ebias", [128, 1], f32, kind="ExternalInput")
    acc_out = nc.dram_tensor("acc_out", [128, NB], f32, kind="ExternalOutput")
    fin_out = nc.dram_tensor("fin_out", [128, NB], bf16, kind="ExternalOutput")

    resc_steps = {W - 1} | {W - 1 + RESC * j for j in range(1, SPAN // RESC + 1)
                            if W - 1 + RESC * j < SPAN}

    with tile.TileContext(nc) as tc:
        with ExitStack() as ctx:
            cpool = ctx.enter_context(tc.tile_pool(name="const", bufs=1))
            rpool = ctx.enter_context(tc.tile_pool(name="rp", bufs=NWIN))
            mpool = ctx.enter_context(tc.tile_pool(name="mp", bufs=2))
            pspool = ctx.enter_context(
                tc.tile_pool(name="ps", bufs=2, space=bass.MemorySpace.PSUM))

            w_t = cpool.tile([128, 128], bf16, tag="w")
            nc.sync.dma_start(w_t[:], wmat[:])
            o_t = cpool.tile([128, 128], bf16, tag="o")
            nc.sync.dma_start(o_t[:], omat[:])
            eb_t = cpool.tile([128, 1], f32, tag="eb")
            nc.sync.dma_start(eb_t[:], ebias[:])

            S, A = [], []
            for g in range(G):
                st = cpool.tile([128, F], bf16, tag=f"S{g}")
                nc.vector.memset(st[:], 1.0)
                ac = cpool.tile([128, F], f32, tag=f"A{g}")
                nc.vector.memset(ac[:], 0.0)
                S.append(st)
                A.append(ac)

            # Load + exp windows.  R[g][w] layout: [128, F, SBLK], partition
            # p = 32*q + k holds chain (g*F + cb)*4 + q, state k.
            R = [[None] * NWIN for _ in range(G)]
            NCHUNK = 4
            CH = F // NCHUNK
            for w in range(NWIN):
                for g in range(G):
                    rt = rpool.tile([128, F, SBLK], f32, tag=f"R{g}")
                    for ch in range(NCHUNK):
                        cb0 = ch * CH
                        for q in range(4):
                            off = ((g * F + cb0) * 4 + q) * L + w * SBLK
                            src = bass.AP(x, off,
                                          [[NSL, 32], [4 * L, CH], [1, SBLK]])
                            nc.sync.dma_start(
                                rt[32 * q:32 * q + 32, cb0:cb0 + CH, :], src)
                    # exp in place, chunked along s so compute starts early
                    EC = 4
                    for ec in range(EC):
                        s0 = ec * (SBLK // EC)
                        nc.scalar.activation(
                            rt[:, :, s0:s0 + SBLK // EC],
                            rt[:, :, s0:s0 + SBLK // EC], AF.Exp,
                            bias=eb_t[:])
                    R[g][w] = rt

            for s in range(SPAN):
                w, si = divmod(s, SBLK)
                for g in range(G):
                    ps = pspool.tile([128, F], f32, tag=f"mm{g}")
                    nc.tensor.matmul(ps[:], w_t[:], S[g][:], start=True, stop=True)
                    nc.vector.tensor_mul(S[g][:], ps[:], R[g][w][:, :, si])
                if s in resc_steps:
                    for g in range(G):
                        sm = pspool.tile([128, F], f32, tag=f"sm{g}")
                        nc.tensor.matmul(sm[:], o_t[:], S[g][:], start=True, stop=True)
                        rc = mpool.tile([128, F], f32, tag=f"rc{g}")
                        nc.vector.reciprocal(rc[:], sm[:])
                        nc.vector.tensor_mul(S[g][:], S[g][:], rc[:])
                        if s >= W:
                            lg = mpool.tile([128, F], f32, tag=f"lg{g}")
                            nc.scalar.activation(lg[:], sm[:], AF.Ln)
                            nc.vector.tensor_add(A[g][:], A[g][:], lg[:])

            for g in range(G):
                nc.sync.dma_start(acc_out[:, g * F:(g + 1) * F], A[g][:])
                nc.sync.dma_start(fin_out[:, g * F:(g + 1) * F], S[g][:])

    nc.compile()
    return nc


def _get_nc():
    if "nc" not in _cache:
        _cache["nc"] = _build()
    return _cache["nc"]


def _log_softmax64(v, axis):
    v = v.astype(np.float64)
    m = v.max(axis=axis, keepdims=True)
    e = np.exp(v - m)
    return v - m - np.log(e.sum(axis=axis, keepdims=True))


def _make_in_maps(log_pdf, T64):
    from ml_dtypes import bfloat16

    T32 = T64.astype(np.float32)
    Tbf = T32.astype(bfloat16)
    # bf16-quantized T is exactly D_r @ T_hat with T_hat row-stochastic and
    # r the bf16 row sums; cancel the bias by folding -log(r) into the exp.
    r = Tbf.astype(np.float64).sum(axis=1)
    eb = np.zeros((128, 1), dtype=np.float32)
    for q in range(4):
        eb[32 * q:32 * q + 32, 0] = (-np.log(r)).astype(np.float32)
    wm = np.zeros((128, 128), dtype=bfloat16)
    om = np.zeros((128, 128), dtype=bfloat16)
    for q in range(4):
        wm[32 * q:32 * q + 32, 32 * q:32 * q + 32] = Tbf.T
        om[32 * q:32 * q + 32, 32 * q:32 * q + 32] = bfloat16(1.0)
    in_maps = []
    for k in range(NCORES):
        c0 = k * CC * L
        in_maps.append({
            "x": np.ascontiguousarray(log_pdf[:, c0:c0 + NSL]),
            "wmat": wm,
            "omat": om,
            "ebias": eb,
        })

    return in_maps


def kernel(log_pdf: np.ndarray, pi: np.ndarray, T: np.ndarray) -> np.ndarray:
    from concourse.bass_utils import run_bass_kernel_spmd

    log_pdf = np.ascontiguousarray(log_pdf, dtype=np.float32)
    log_pi64 = _log_softmax64(pi, 0)
    log_T64 = _log_softmax64(T, 1)
    T64 = np.exp(log_T64)                     # row-stochastic [K, K] f64

    in_maps = _make_in_maps(log_pdf, T64)
    nc = _get_nc()
    res = run_bass_kernel_spmd(nc, in_maps, list(range(NCORES))).results

    # ---- host combine (f64) ----
    LP = log_pdf  # f32 view; cast per-column below
    # exact prefix [0, W)
    a = np.exp(log_pi64 + LP[:, 0].astype(np.float64))
    c = a.sum()
    total = np.log(c)
    a /= c
    for t in range(1, W):
        a = np.exp(LP[:, t].astype(np.float64)) * (T64 @ a)
        c = a.sum()
        total += np.log(c)
        a /= c

    # per-chain contributions
    last_vec = None
    for k in range(NCORES):
        acc = res[k]["acc_out"]   # [128, NB]
        fin = res[k]["fin_out"]   # [128, NB]
        for g in range(G):
            for cb in range(F):
                for q in range(4):
                    col = g * F + cb
                    vec = fin[32 * q:32 * q + 32, col].astype(np.float64)
                    sv = vec.sum()
                    total += acc[32 * q, col].astype(np.float64) + np.log(sv)
                    if (k == NCORES - 1 and g == G - 1 and cb == F - 1
                            and q == 3):
                        last_vec = vec / sv

    # exact tail [COVERED, N)
    a = last_vec
    for t in range(COVERED, N):
        a = np.exp(LP[:, t].astype(np.float64)) * (T64 @ a)
        c = a.sum()
        total += np.log(c)
        a /= c

    return np.float32(total)


# revision 9
# speedup vs baseline: 1.5726x; 1.0130x over previous
"""HMM log-likelihood (log-domain forward algorithm) on 8 Trainium2 cores.

Strategy: scaled linear-domain forward algorithm with warmup-halo sequence
parallelism.  The filtering distribution of an HMM forgets its initial
condition geometrically fast, so N=1e6 timesteps are split into 3840
independent chains (480/core); each chain starts from a uniform state W=20
steps before its owned region of L=260 steps.  Per core, chains are batched
4-wide across the 128 SBUF partitions (block-diagonal T^T weights on the PE)
with the chain-block index in the matmul free dimension, so each timestep is
one bf16 matmul (T @ S into PSUM) plus one vector multiply by the emission
probabilities.

Normalization is free: a constant per-step drift delta = E[log c] is folded
into the exp bias, making log|S| a zero-drift random walk (~26 bits 4.5
sigma over a 280-step chain — far inside f32 range), so the kernel needs no
per-chain rescaling.  The bf16 quantization of T factors exactly as
D_r @ T_hat with T_hat row-stochastic; -log(r) is folded into the same exp
bias.  Each chain's contribution is log(sum(S_final)) - log(sum(S_at_W)) +
delta*L, assembled on the host, which also runs exact f64 scans for the
prefix [0, W) and the short tail.
"""

import sys

for p in ("/opt/trn_rl_repo", "/root/.axon_site", "/root/.axon_site/_ro/trn_rl_repo",
          "/root/.axon_site/_ro/pypackages"):
    if p not in sys.path:
        sys.path.insert(0, p)

import numpy as np

K = 32
N = 1_000_000
NCORES = 8
W = 20            # warmup (halo) steps per chain
L = 260           # owned steps per chain
CC = 480          # chains per core
SPAN = W + L      # 280 sequential steps
SBLK = 140        # timesteps per load window
NWIN = SPAN // SBLK
NB = CC // 4      # 120 four-chain blocks
G = 2             # interleaved compute groups
F = NB // G       # 60 blocks (matmul free dim) per group
NSL = CC * L + W  # per-core input slice columns
COVERED = W + NCORES * CC * L

_cache = {}


def _build():
    import concourse.bass as bass
    import concourse.bacc as bacc
    import concourse.mybir as mybir
    import concourse.tile as tile
    from contextlib import ExitStack

    f32 = mybir.dt.float32
    bf16 = mybir.dt.bfloat16
    AF = mybir.ActivationFunctionType

    nc = bacc.Bacc("TRN2", target_bir_lowering=False, debug=False,
                   num_devices=NCORES)
    x = nc.dram_tensor("x", [K, NSL], f32, kind="ExternalInput")
    wmat = nc.dram_tensor("wmat", [128, 128], bf16, kind="ExternalInput")
    ebias = nc.dram_tensor("ebias", [128, 1], f32, kind="ExternalInput")
    snap_out = nc.dram_tensor("snap_out", [128, NB], bf16, kind="ExternalOutput")
    fin_out = nc.dram_tensor("fin_out", [128, NB], bf16, kind="ExternalOutput")

    with tile.TileContext(nc) as tc:
        with ExitStack() as ctx:
            cpool = ctx.enter_context(tc.tile_pool(name="const", bufs=1))
            rpool = ctx.enter_context(tc.tile_pool(name="rp", bufs=NWIN))
            pspool = ctx.enter_context(
                tc.tile_pool(name="ps", bufs=2, space=bass.MemorySpace.PSUM))

            w_t = cpool.tile([128, 128], bf16, tag="w")
            nc.sync.dma_start(w_t[:], wmat[:])
            eb_t = cpool.tile([128, 1], f32, tag="eb")
            nc.sync.dma_start(eb_t[:], ebias[:])

            S, SN = [], []
            for g in range(G):
                st = cpool.tile([128, F], bf16, tag=f"S{g}")
                nc.vector.memset(st[:], 1.0)
                sn = cpool.tile([128, F], bf16, tag=f"N{g}")
                S.append(st)
                SN.append(sn)

            # Load + exp windows.  R[g][w] layout: [128, F, SBLK], partition
            # p = 32*q + k holds chain (g*F + cb)*4 + q, state k.
            R = [[None] * NWIN for _ in range(G)]
            NCHUNK = 4
            CH = F // NCHUNK
            for w in range(NWIN):
                for g in range(G):
                    rt = rpool.tile([128, F, SBLK], f32, tag=f"R{g}")
                    for ch in range(NCHUNK):
                        cb0 = ch * CH
                        for q in range(4):
                            off = ((g * F + cb0) * 4 + q) * L + w * SBLK
                            src = bass.AP(x, off,
                                          [[NSL, 32], [4 * L, CH], [1, SBLK]])
                            nc.sync.dma_start(
                                rt[32 * q:32 * q + 32, cb0:cb0 + CH, :], src)
                    # exp in place, chunked along s so compute starts early
                    EC = 4
                    for ec in range(EC):
                        s0 = ec * (SBLK // EC)
                        nc.scalar.activation(
                            rt[:, :, s0:s0 + SBLK // EC],
                            rt[:, :, s0:s0 + SBLK // EC], AF.Exp,
                            bias=eb_t[:])
                    R[g][w] = rt

            for s in range(SPAN):
                w, si = divmod(s, SBLK)
                for g in range(G):
                    ps = pspool.tile([128, F], f32, tag=f"mm{g}")
                    nc.tensor.matmul(ps[:], w_t[:], S[g][:], start=True, stop=True)
                    nc.vector.tensor_mul(S[g][:], ps[:], R[g][w][:, :, si])
                    if s == W - 1:
                        nc.vector.tensor_copy(SN[g][:], S[g][:])

            for g in range(G):
                nc.sync.dma_start(snap_out[:, g * F:(g + 1) * F], SN[g][:])
                nc.sync.dma_start(fin_out[:, g * F:(g + 1) * F], S[g][:])

    nc.compile()
    return nc


def _get_nc():
    if "nc" not in _cache:
        _cache["nc"] = _build()
    return _cache["nc"]


def _log_softmax64(v, axis):
    v = v.astype(np.float64)
    m = v.max(axis=axis, keepdims=True)
    e = np.exp(v - m)
    return v - m - np.log(e.sum(axis=axis, keepdims=True))


def _estimate_delta(log_pdf, T64):
    # E[log c] from a vectorized short scan: 64 parallel probes, 56 steps,
    # burn-in 16 (mixing time is ~10 steps).
    NCH, NST, BURN = 64, 56, 16
    cols = np.arange(NCH) * 997 + 1
    a = np.full((K, NCH), 1.0 / K)
    samples = []
    for s in range(NST):
        p = np.exp(log_pdf[:, cols + s].astype(np.float64))
        a = p * (T64 @ a)
        c = a.sum(axis=0)
        a /= c
        if s >= BURN:
            samples.append(np.log(c))
    return float(np.mean(samples))


def _make_in_maps(log_pdf, T64):
    from ml_dtypes import bfloat16

    T32 = T64.astype(np.float32)
    Tbf = T32.astype(bfloat16)
    delta = _estimate_delta(log_pdf, T64)
    # bf16-quantized T is exactly D_r @ T_hat with T_hat row-stochastic and
    # r the bf16 row sums; fold -log(r) and the drift -delta into the exp.
    r = Tbf.astype(np.float64).sum(axis=1)
    eb = np.zeros((128, 1), dtype=np.float32)
    for q in range(4):
        eb[32 * q:32 * q + 32, 0] = (-np.log(r) - delta).astype(np.float32)
    wm = np.zeros((128, 128), dtype=bfloat16)
    for q in range(4):
        wm[32 * q:32 * q + 32, 32 * q:32 * q + 32] = Tbf.T
    in_maps = []
    for k in range(NCORES):
        c0 = k * CC * L
        in_maps.append({
            "x": np.ascontiguousarray(log_pdf[:, c0:c0 + NSL]),
            "wmat": wm,
            "ebias": eb,
        })

    return in_maps, delta


def kernel(log_pdf: np.ndarray, pi: np.ndarray, T: np.ndarray) -> np.ndarray:
    from concourse.bass_utils import run_bass_kernel_spmd

    log_pdf = np.ascontiguousarray(log_pdf, dtype=np.float32)
    log_pi64 = _log_softmax64(pi, 0)
    log_T64 = _log_softmax64(T, 1)
    T64 = np.exp(log_T64)                     # row-stochastic [K, K] f64

    in_maps, delta = _make_in_maps(log_pdf, T64)
    nc = _get_nc()
    res = run_bass_kernel_spmd(nc, in_maps, list(range(NCORES))).results

    # ---- host combine (f64) ----
    LP = log_pdf
    # exact prefix [0, W)
    a = np.exp(log_pi64 + LP[:, 0].astype(np.float64))
    c = a.sum()
    total = np.log(c)
    a /= c
    for t in range(1, W):
        a = np.exp(LP[:, t].astype(np.float64)) * (T64 @ a)
        c = a.sum()
        total += np.log(c)
        a /= c

    # per-chain contributions: log(sum fin) - log(sum snap) + delta*L
    for k in range(NCORES):
        snap = res[k]["snap_out"].astype(np.float64)   # [128, NB]
        fin = res[k]["fin_out"].astype(np.float64)
        for q in range(4):
            ssum = snap[32 * q:32 * q + 32, :].sum(axis=0)
            fsum = fin[32 * q:32 * q + 32, :].sum(axis=0)
            total += (np.log(fsum) - np.log(ssum)).sum() + delta * L * NB

    # exact tail [COVERED, N) from the last chain's final state
    k, g, cb, q = NCORES - 1, G - 1, F - 1, 3
    fv = res[k]["fin_out"][32 * q:32 * q + 32, g * F + cb].astype(np.float64)
    a = fv / fv.sum()
    for t in range(COVERED, N):
        a = np.exp(LP[:, t].astype(np.float64)) * (T64 @ a)
        c = a.sum()
        total += np.log(c)
        a /= c

    return np.float32(total)


# revision 11
# speedup vs baseline: 2.1031x; 1.3373x over previous
"""HMM log-likelihood (log-domain forward algorithm) on 8 Trainium2 cores.

Strategy: scaled linear-domain forward algorithm with warmup-halo sequence
parallelism.  The filtering distribution of an HMM forgets its initial
condition geometrically fast, so N=1e6 timesteps are split into 3840
independent chains (480/core); each chain starts from a uniform state W=20
steps before its owned region of L=260 steps.  Per core, chains are batched
4-wide across the 128 SBUF partitions (block-diagonal T^T weights on the PE)
with the chain-block index in the matmul free dimension, so each timestep is
one bf16 matmul (T @ S into PSUM) plus one vector multiply by the emission
probabilities.

Normalization is free: a constant per-step drift delta = E[log c] is folded
into the exp bias, making log|S| a zero-drift random walk (~26 bits 4.5
sigma over a 280-step chain — far inside f32 range), so the kernel needs no
per-chain rescaling.  The bf16 quantization of T factors exactly as
D_r @ T_hat with T_hat row-stochastic; -log(r) is folded into the same exp
bias.  Each chain's contribution is log(sum(S_final)) - log(sum(S_at_W)) +
delta*L, assembled on the host, which also runs exact f64 scans for the
prefix [0, W) and the short tail.
"""

import sys

for p in ("/opt/trn_rl_repo", "/root/.axon_site", "/root/.axon_site/_ro/trn_rl_repo",
          "/root/.axon_site/_ro/pypackages"):
    if p not in sys.path:
        sys.path.insert(0, p)

import numpy as np

K = 32
N = 1_000_000
NCORES = 8
W = 20            # warmup (halo) steps per chain
L = 260           # owned steps per chain
CC = 480          # chains per core
SPAN = W + L      # 280 sequential steps
SBLK = 140        # timesteps per load window
NWIN = SPAN // SBLK
NB = CC // 4      # 120 four-chain blocks
G = 2             # interleaved compute groups
F = NB // G       # 60 blocks (matmul free dim) per group
NSL = CC * L + W  # per-core input slice columns
COVERED = W + NCORES * CC * L

_cache = {}


def _build():
    import concourse.bass as bass
    import concourse.bacc as bacc
    import concourse.mybir as mybir
    import concourse.tile as tile
    from contextlib import ExitStack

    f32 = mybir.dt.float32
    bf16 = mybir.dt.bfloat16
    AF = mybir.ActivationFunctionType

    nc = bacc.Bacc("TRN2", target_bir_lowering=False, debug=False,
                   num_devices=NCORES)
    x = nc.dram_tensor("x", [K, NSL], f32, kind="ExternalInput")
    wmat = nc.dram_tensor("wmat", [128, 128], bf16, kind="ExternalInput")
    ebias = nc.dram_tensor("ebias", [128, 1], f32, kind="ExternalInput")
    snap_out = nc.dram_tensor("snap_out", [128, NB], bf16, kind="ExternalOutput")
    fin_out = nc.dram_tensor("fin_out", [128, NB], bf16, kind="ExternalOutput")

    with tile.TileContext(nc) as tc:
        with ExitStack() as ctx:
            cpool = ctx.enter_context(tc.tile_pool(name="const", bufs=1))
            rpool = ctx.enter_context(tc.tile_pool(name="rp", bufs=NWIN))
            pspool = ctx.enter_context(
                tc.tile_pool(name="ps", bufs=2, space=bass.MemorySpace.PSUM))

            w_t = cpool.tile([128, 128], bf16, tag="w")
            nc.sync.dma_start(w_t[:], wmat[:])
            eb_t = cpool.tile([128, 1], f32, tag="eb")
            nc.sync.dma_start(eb_t[:], ebias[:])

            S, SN = [], []
            for g in range(G):
                st = cpool.tile([128, F], bf16, tag=f"S{g}")
                nc.vector.memset(st[:], 1.0)
                sn = cpool.tile([128, F], bf16, tag=f"N{g}")
                S.append(st)
                SN.append(sn)

            # Load + exp windows.  R[g][w] layout: [128, F, SBLK], partition
            # p = 32*q + k holds chain (g*F + cb)*4 + q, state k.
            R = [[None] * NWIN for _ in range(G)]
            NCHUNK = 4
            CH = F // NCHUNK
            # interleave DMA chunks and exp chunks across groups so both
            # chains become runnable at the same (early) time
            for w in range(NWIN):
                for g in range(G):
                    rt = rpool.tile([128, F, SBLK], f32, tag=f"R{g}",
                                    name=f"rt{g}_{w}")
                    R[g][w] = rt
                for ch in range(NCHUNK):
                    for g in range(G):
                        rt = R[g][w]
                        cb0 = ch * CH
                        for q in range(4):
                            off = ((g * F + cb0) * 4 + q) * L + w * SBLK
                            src = bass.AP(x, off,
                                          [[NSL, 32], [4 * L, CH], [1, SBLK]])
                            nc.sync.dma_start(
                                rt[32 * q:32 * q + 32, cb0:cb0 + CH, :], src)
                # exp in place, chunked along s so compute starts early
                EC = 7
                for ec in range(EC):
                    for g in range(G):
                        rt = R[g][w]
                        s0 = ec * (SBLK // EC)
                        nc.scalar.activation(
                            rt[:, :, s0:s0 + SBLK // EC],
                            rt[:, :, s0:s0 + SBLK // EC], AF.Exp,
                            bias=eb_t[:])

            for s in range(SPAN):
                w, si = divmod(s, SBLK)
                for g in range(G):
                    ps = pspool.tile([128, F], f32, tag=f"mm{g}")
                    nc.tensor.matmul(ps[:], w_t[:], S[g][:], start=True, stop=True)
                    nc.vector.tensor_mul(S[g][:], ps[:], R[g][w][:, :, si])
                    if s == W - 1:
                        nc.vector.tensor_copy(SN[g][:], S[g][:])

            for g in range(G):
                nc.sync.dma_start(snap_out[:, g * F:(g + 1) * F], SN[g][:])
                nc.sync.dma_start(fin_out[:, g * F:(g + 1) * F], S[g][:])

    nc.compile()
    return nc


def _get_nc():
    if "nc" not in _cache:
        _cache["nc"] = _build()
    return _cache["nc"]


def _log_softmax64(v, axis):
    v = v.astype(np.float64)
    m = v.max(axis=axis, keepdims=True)
    e = np.exp(v - m)
    return v - m - np.log(e.sum(axis=axis, keepdims=True))


def _estimate_delta(log_pdf, T64):
    # E[log c] from a vectorized short scan: 64 parallel probes, 56 steps,
    # burn-in 16 (mixing time is ~10 steps).
    NCH, NST, BURN = 64, 56, 16
    cols = np.arange(NCH) * 997 + 1
    a = np.full((K, NCH), 1.0 / K)
    samples = []
    for s in range(NST):
        p = np.exp(log_pdf[:, cols + s].astype(np.float64))
        a = p * (T64 @ a)
        c = a.sum(axis=0)
        a /= c
        if s >= BURN:
            samples.append(np.log(c))
    return float(np.mean(samples))


def _make_in_maps(log_pdf, T64):
    from ml_dtypes import bfloat16

    T32 = T64.astype(np.float32)
    Tbf = T32.astype(bfloat16)
    delta = _estimate_delta(log_pdf, T64)
    # bf16-quantized T is exactly D_r @ T_hat with T_hat row-stochastic and
    # r the bf16 row sums; fold -log(r) and the drift -delta into the exp.
    r = Tbf.astype(np.float64).sum(axis=1)
    eb = np.zeros((128, 1), dtype=np.float32)
    for q in range(4):
        eb[32 * q:32 * q + 32, 0] = (-np.log(r) - delta).astype(np.float32)
    wm = np.zeros((128, 128), dtype=bfloat16)
    for q in range(4):
        wm[32 * q:32 * q + 32, 32 * q:32 * q + 32] = Tbf.T
    in_maps = []
    for k in range(NCORES):
        c0 = k * CC * L
        in_maps.append({
            "x": np.ascontiguousarray(log_pdf[:, c0:c0 + NSL]),
            "wmat": wm,
            "ebias": eb,
        })

    return in_maps, delta


def kernel(log_pdf: np.ndarray, pi: np.ndarray, T: np.ndarray) -> np.ndarray:
    from concourse.bass_utils import run_bass_kernel_spmd

    log_pdf = np.ascontiguousarray(log_pdf, dtype=np.float32)
    log_pi64 = _log_softmax64(pi, 0)
    log_T64 = _log_softmax64(T, 1)
    T64 = np.exp(log_T64)                     # row-stochastic [K, K] f64

    in_maps, delta = _make_in_maps(log_pdf, T64)
    nc = _get_nc()
    res = run_bass_kernel_spmd(nc, in_maps, list(range(NCORES))).results

    # ---- host combine (f64) ----
    LP = log_pdf
    # exact prefix [0, W)
    a = np.exp(log_pi64 + LP[:, 0].astype(np.float64))
    c = a.sum()
    total = np.log(c)
    a /= c
    for t in range(1, W):
        a = np.exp(LP[:, t].astype(np.float64)) * (T64 @ a)
        c = a.sum()
        total += np.log(c)
        a /= c

    # per-chain contributions: log(sum fin) - log(sum snap) + delta*L
    for k in range(NCORES):
        snap = res[k]["snap_out"].astype(np.float64)   # [128, NB]
        fin = res[k]["fin_out"].astype(np.float64)
        for q in range(4):
            ssum = snap[32 * q:32 * q + 32, :].sum(axis=0)
            fsum = fin[32 * q:32 * q + 32, :].sum(axis=0)
            total += (np.log(fsum) - np.log(ssum)).sum() + delta * L * NB

    # exact tail [COVERED, N) from the last chain's final state
    k, g, cb, q = NCORES - 1, G - 1, F - 1, 3
    fv = res[k]["fin_out"][32 * q:32 * q + 32, g * F + cb].astype(np.float64)
    a = fv / fv.sum()
    for t in range(COVERED, N):
        a = np.exp(LP[:, t].astype(np.float64)) * (T64 @ a)
        c = a.sum()
        total += np.log(c)
        a /= c

    return np.float32(total)
